# revision 26
# baseline (speedup 1.0000x reference)
"""Trainium2 Bass kernel for nn_AttentionOperation_32521492365427.

kernel(**inputs) -> np.ndarray, full shapes:
  query/key/value: [8, 8, 64, 1024] f32; gamma_sim/beta_sim: [8];
  gamma_val/beta_val: [512]; output: [8, 512, 1024] f32.

Sharded by HEAD across the 8 NeuronCores (one head per core): both
BatchNorms then have core-local statistics, so there are no collectives.

Per-core math:
 - softmax is shift-invariant => the sim-BN reduces to one per-head scale
   s = gamma_sim / sqrt(var(logits) + EPS); beta/mean drop out.
 - sumsq(logits_b) = sum(Gq_b * Gk_b) over 65x65 Gram matrices
   Gq = Q1^T Q1 (ones column appended) => logits variance without a
   stats pass over the 8.4M logits.
 - softmax denominators come free as row 0 of the PV matmul by having a
   ones column at position 0 of V^T (the stationary operand).
 - val-BN affine + exact (erf) gelu fuse into a single ACT pass.

v2 changes vs baseline:
 - all matmul operands fp16 (1 cyc/row like bf16, 4x the mantissa);
   host pre-converts so no DMA dtype conversion.
 - denominator broadcast via gpsimd partition_broadcast instead of an
   fp32 ones-matmul (fp32 matmuls are 4 cyc/row and burn PSUM banks).
 - all inputs SBUF-resident; per-batch DMA slices overlap phase 0.
 - PSUM: 2x lg [128,1024] + 2x pv [65,1024] = all 8 banks.
"""

import os
import sys

sys.path.insert(0, "/opt/trn_rl_repo")

from contextlib import ExitStack

import numpy as np

import concourse.bacc as bacc
import concourse.bass as bass  # noqa: F401
import concourse.tile as tile
from concourse import mybir

F32 = mybir.dt.float32
F16 = mybir.dt.float16
BF16 = mybir.dt.bfloat16
I32 = mybir.dt.int32
I16 = mybir.dt.int16
AF = mybir.ActivationFunctionType
OP = mybir.AluOpType

EPS = 1e-3
NB = 8
D = 64
C = 64
L = 1024
M = 1024
NCH = M // 128
NLM = float(NB * L * M)
MAGIC = 0x5F3759DF
# DVE Schraudolph exp: bf16 bits = int16(x * 128/ln2 + 16250.49); ~3% max rel
# err, exact dynamic range to e^+-88. Offloaded chunks relieve the ACT engine.
SCHRAU_A = 184.6635
SCHRAU_B = 16250.49
DVE_EXP_CHUNKS = (5,)


def _newton_rsqrt(nc, x, y, t, magic_i32, iters=1):
    """y = 1/sqrt(x) entirely on DVE (bit-trick seed + Newton iters)."""
    xi = x.bitcast(I32)
    yi = y.bitcast(I32)
    nc.vector.tensor_scalar(
        out=yi, in0=xi, scalar1=1, scalar2=None, op0=OP.arith_shift_right
    )
    nc.vector.tensor_tensor(out=yi, in0=magic_i32, in1=yi, op=OP.subtract)
    for _ in range(iters):
        nc.vector.tensor_mul(t, y, y)
        nc.vector.tensor_mul(t, t, x)
        nc.vector.tensor_scalar(
            out=t, in0=t, scalar1=-0.5, scalar2=1.5, op0=OP.mult, op1=OP.add
        )
        nc.vector.tensor_mul(y, y, t)


def _dedupe_ldweights(nc):
    """Remove back-to-back InstLdweights whose weights AP is identical to the
    one already resident in the PE array (warmup repeats and the QK/PV
    j-pairs). Waits on a removed load are merged into its matmul."""
    removed = 0
    for bb in nc.m.functions[0].blocks:
        insts = list(bb.instructions)
        out = []
        last_sig = None
        k = 0
        while k < len(insts):
            inst = insts[k]
            tname = type(inst).__name__
            if tname == "InstLdweights":
                ap = inst.ins[0]
                sig = (
                    str(ap),
                    getattr(inst, "perf_mode", None),
                    getattr(inst, "is_transpose", None),
                    getattr(inst, "tile_position", None),
                )
                nxt = insts[k + 1] if k + 1 < len(insts) else None
                if (
                    sig == last_sig
                    and nxt is not None
                    and type(nxt).__name__ == "InstMatmult"
                ):
                    si = inst.sync_info
                    if si is not None and si.on_wait:
                        nsi = nxt.sync_info
                        if nsi is None:
                            nxt.sync_info = si
                        else:
                            nsi.on_wait.extend(si.on_wait)
                            nxt.sync_info = nsi
                    if si is not None and si.on_update:
                        nsi = nxt.sync_info
                        nsi.on_update.extend(si.on_update)
                        nxt.sync_info = nsi
                    removed += 1
                    k += 1
                    continue
                last_sig = sig
            elif tname == "InstMatmult":
                pass  # matmuls do not disturb the loaded stationary
            elif getattr(inst, "engine", None) == mybir.EngineType.PE and not getattr(
                inst, "is_sequencer_only", False
            ):
                last_sig = None  # unknown PE instruction: assume clobbered
            out.append(inst)
            k += 1
        if removed:
            bb.instructions = out
    return removed


def build_nc(debug: bool = False):
    nc = bacc.Bacc("TRN2", target_bir_lowering=False, debug=debug)

    q2_d = nc.dram_tensor("q2", [128, NB // 2, L], F16, kind="ExternalInput")
    k2_d = nc.dram_tensor("k2", [128, NB // 2, L], F16, kind="ExternalInput")
    qt1_d = nc.dram_tensor("qt1", [128, NB, NCH, 65], F16, kind="ExternalInput")
    kt1_d = nc.dram_tensor("kt1", [128, NB, NCH, 65], F16, kind="ExternalInput")
    vt1_d = nc.dram_tensor("vt1", [128, NB, NCH, 65], BF16, kind="ExternalInput")
    gsim_d = nc.dram_tensor("g_sim", [1, 1], F32, kind="ExternalInput")
    gval_d = nc.dram_tensor("gamma_val", [C, 1], F32, kind="ExternalInput")
    bval_d = nc.dram_tensor("beta_val", [C, 1], F32, kind="ExternalInput")
    out_d = nc.dram_tensor("out", [NB, C, L], F32, kind="ExternalOutput")

    with tile.TileContext(nc) as tc, ExitStack() as ctx:
        const_p = ctx.enter_context(tc.tile_pool(name="const", bufs=1))
        pt_p = ctx.enter_context(tc.tile_pool(name="pt", bufs=3))
        dn_p = ctx.enter_context(tc.tile_pool(name="dn", bufs=2))
        big_p = ctx.enter_context(tc.tile_pool(name="big", bufs=1))
        small = ctx.enter_context(tc.tile_pool(name="small", bufs=1))

        # ---- constants + resident inputs ----
        ones65 = const_p.tile([65, 1], F32, tag="ones65")
        nc.vector.memset(ones65[:], 1.0)
        warm_w = const_p.tile([128, 128], F16, tag="warmw")
        nc.vector.memset(warm_w[:], 1.0)
        warm_m = const_p.tile([128, 512], F16, tag="warmm")
        nc.vector.memset(warm_m[:], 1.0)
        magic_sb = const_p.tile([65, 1], I32, tag="magic")
        nc.vector.memset(magic_sb[:], MAGIC)
        gsim_sb = const_p.tile([1, 1], F32, tag="gsim")
        nc.sync.dma_start(out=gsim_sb[:], in_=gsim_d[:])
        gval_sb = const_p.tile([C, 1], F32, tag="gval")
        nc.sync.dma_start(out=gval_sb[:], in_=gval_d[:])
        bval_sb = const_p.tile([C, 1], F32, tag="bval")
        nc.sync.dma_start(out=bval_sb[:], in_=bval_d[:])

        qt_sb = const_p.tile([128, NB, NCH, 65], F16, tag="qt")
        kt_sb = const_p.tile([128, NB, NCH, 65], F16, tag="kt")
        vt_sb = const_p.tile([128, NB, NCH, 65], BF16, tag="vt")
        q2_sb = const_p.tile([128, NB // 2, L], F16, tag="q2")
        k2_sb = const_p.tile([128, NB // 2, L], F16, tag="k2")
        # per-batch slices so phase-0 consumers start as data lands
        for b in range(NB):
            nc.gpsimd.dma_start(out=qt_sb[:, b], in_=qt1_d[:, b])
            nc.sync.dma_start(out=kt_sb[:, b], in_=kt1_d[:, b])
        for p in range(NB // 2):
            nc.scalar.dma_start(out=q2_sb[:, p, :], in_=q2_d[:, p, :])
            nc.sync.dma_start(out=k2_sb[:, p, :], in_=k2_d[:, p, :])
        for b in range(NB):
            nc.gpsimd.dma_start(out=vt_sb[:, b], in_=vt1_d[:, b])

        s_bcast = small.tile([128, 1], F32, tag="sbc")

        # ---- phase 0: Gram-matrix logits variance -> s ----
        with tc.tile_pool(name="gram", bufs=2, space="PSUM") as gram_p, tc.tile_pool(
            name="sf", bufs=1, space="PSUM"
        ) as sf_p, tc.tile_pool(name="warm", bufs=1, space="PSUM") as warm_p:
            wps = warm_p.tile([128, 512], F32, tag="warm")
            for _ in range(2):
                nc.tensor.matmul(wps[:], warm_w[:], warm_m[:], start=True, stop=True)

            acc = small.tile([65, NB], F32, tag="acc")
            for b in range(NB):
                gq_ps = gram_p.tile([65, 65], F32, tag="gq")
                gk_ps = gram_p.tile([65, 65], F32, tag="gk")
                for c in range(NCH):
                    nc.tensor.matmul(
                        gq_ps[:],
                        qt_sb[:, b, c, :],
                        qt_sb[:, b, c, :],
                        start=(c == 0),
                        stop=(c == NCH - 1),
                    )
                for c in range(NCH):
                    nc.tensor.matmul(
                        gk_ps[:],
                        kt_sb[:, b, c, :],
                        kt_sb[:, b, c, :],
                        start=(c == 0),
                        stop=(c == NCH - 1),
                    )
                gq_sb = small.tile([65, 65], F32, tag="gq_sb")
                nc.vector.tensor_copy(gq_sb[:], gq_ps[:])
                prod = small.tile([65, 65], F32, tag="prod")
                nc.vector.tensor_mul(prod[:], gq_sb[:], gk_ps[:])
                nc.vector.reduce_sum(
                    acc[:, b : b + 1], prod[:, 0:64], axis=mybir.AxisListType.X
                )

            red = small.tile([65, 1], F32, tag="red")
            nc.vector.reduce_sum(red[:], acc[:], axis=mybir.AxisListType.X)
            rhs65 = small.tile([65, 1], F32, tag="rhs65")
            nc.vector.tensor_scalar_mul(rhs65[0:64, :], red[0:64, :], 1.0 / NLM)
            nc.vector.tensor_scalar_mul(rhs65[64:65, :], red[64:65, :], 1.0 / NLM)
            nc.vector.tensor_mul(rhs65[64:65, :], rhs65[64:65, :], rhs65[64:65, :])
            nc.vector.tensor_scalar_mul(rhs65[64:65, :], rhs65[64:65, :], -1.0)
            var_ps = sf_p.tile([1, 1], F32, tag="var")
            nc.tensor.matmul(
                var_ps[:], ones65[:], rhs65[:], start=True, stop=True
            )
            sv = small.tile([1, 6], F32, tag="sv")
            nc.vector.tensor_scalar_add(sv[:, 0:1], var_ps[:], EPS)
            _newton_rsqrt(nc, sv[:, 0:1], sv[:, 1:2], sv[:, 2:3], magic_sb[0:1, :])
            nc.vector.tensor_mul(sv[:, 3:4], sv[:, 1:2], gsim_sb[:])
            s_src = small.tile([1, 1], F32, tag="s_src")
            nc.vector.tensor_copy(s_src[:], sv[:, 3:4])
            nc.gpsimd.partition_broadcast(s_bcast[:], s_src[:])
        sa_bcast = small.tile([128, 1], F32, tag="sabc")
        nc.vector.tensor_scalar_mul(sa_bcast[:], s_bcast[:], SCHRAU_A)

        # ---- phase A: QK -> exp -> PV -> normalize + stats ----
        # Software-pipelined: PE stream per slot is [PV(s), QK(s+3)] so the
        # tensor engine never waits on the exp of the current chunk.
        ue_sb = big_p.tile([C, NB, L], F32, tag="ue")
        stats = small.tile([C, NB * 2 * 6], F32, tag="stats")

        with tc.tile_pool(name="lg", bufs=3, space="PSUM") as lg_p, tc.tile_pool(
            name="pv", bufs=1, space="PSUM"
        ) as pv_p, tc.tile_pool(name="rv", bufs=2) as rv_p:
            lg_tiles = {}
            pt_tiles = {}
            pv_tiles = {}
            rv_tiles = {}
            dn_tiles = {}
            pending = {}
            NS = NB * NCH

            def at_slot(s, fn):
                pending.setdefault(s, []).append(fn)

            def emit_qk(s):
                b, c = divmod(s, NCH)
                pair, r = divmod(b, 2)
                rs = slice(r * 64, r * 64 + 64)
                lg = lg_p.tile([128, L], F32, tag="lg")
                for j in range(2):
                    nc.tensor.matmul(
                        lg[:, j * 512 : (j + 1) * 512],
                        k2_sb[rs, pair, c * 128 : (c + 1) * 128],
                        q2_sb[rs, pair, j * 512 : (j + 1) * 512],
                        start=True,
                        stop=True,
                    )
                lg_tiles[s] = lg

            def emit_exp(s):
                c = s % NCH
                lg = lg_tiles.pop(s)
                pt = pt_p.tile([128, L], BF16, tag="pt")
                if c in DVE_EXP_CHUNKS:
                    nc.vector.tensor_scalar(
                        out=pt.bitcast(I16)[:],
                        in0=lg[:],
                        scalar1=sa_bcast[:, 0:1],
                        scalar2=SCHRAU_B,
                        op0=OP.mult,
                        op1=OP.add,
                    )
                else:
                    nc.scalar.activation(pt[:], lg[:], AF.Exp, scale=s_bcast[:, 0:1])
                pt_tiles[s] = pt

            # staged pv drain: rv-copy/den row right away (frees pv), the
            # rest spread over later slots so DVE keeps serving the
            # schraudolph chunks without queueing behind the whole chain.
            def drain_a(b):
                pv = pv_tiles.pop(b)
                rv = rv_p.tile([C, L], F32, tag="rv")
                nc.scalar.copy(rv[:], pv[0:C, :])
                rv_tiles[b] = rv
                den0 = dn_p.tile([1, L], F32, tag="den0")
                nc.vector.tensor_copy(den0[:], pv[64:65, :])
                dn_tiles[b] = den0

            def drain_b(b):
                den0 = dn_tiles.pop(b)
                dnb = dn_p.tile([C, L], F32, tag="dnb")
                nc.gpsimd.partition_broadcast(dnb[:], den0[:])
                rec = dn_p.tile([C, L], F32, tag="rec")
                nc.vector.reciprocal_approx_fast(out=rec[:], in_=dnb[:])
                dn_tiles[b] = rec

            def drain_c(b):
                rec = dn_tiles.pop(b)
                rv = rv_tiles.pop(b)
                nc.vector.tensor_mul(ue_sb[:, b, :], rv[:], rec[:])

            def drain_d(b):
                for half in range(2):
                    nc.vector.bn_stats(
                        stats[:, (b * 2 + half) * 6 : (b * 2 + half + 1) * 6],
                        ue_sb[:, b, half * 512 : (half + 1) * 512],
                    )

            for s in range(3):
                emit_qk(s)
            emit_exp(0)
            for s in range(NS):
                b, c = divmod(s, NCH)
                if s + 1 < NS:
                    emit_exp(s + 1)
                if c == 0:
                    pv_tiles[b] = pv_p.tile([65, L], F32, tag="pv", name="pv")
                pv = pv_tiles[b]
                pt = pt_tiles.pop(s)
                for j in range(2):
                    nc.tensor.matmul(
                        pv[:, j * 512 : (j + 1) * 512],
                        vt_sb[:, b, c, :],
                        pt[:, j * 512 : (j + 1) * 512],
                        start=(c == 0),
                        stop=(c == NCH - 1),
                        skip_group_check=True,
                    )
                if s + 3 < NS:
                    emit_qk(s + 3)
                if c == NCH - 1:
                    at_slot(s, lambda bb=b: drain_a(bb))
                    at_slot(s, lambda bb=b: drain_b(bb))
                    at_slot(s, lambda bb=b: drain_c(bb))
                    at_slot(s, lambda bb=b: drain_d(bb))
                for fn in pending.pop(s, []):
                    fn()
            for s in sorted(pending):
                for fn in pending[s]:
                    fn()

        # ---- phase B: val-BN affine + gelu + store ----
        chan = small.tile([C, 2], F32, tag="chan")
        nc.vector.bn_aggr(chan[:], stats[:])
        vb = small.tile([C, 6], F32, tag="vb")
        nc.vector.tensor_scalar_add(vb[:, 0:1], chan[:, 1:2], EPS)
        _newton_rsqrt(nc, vb[:, 0:1], vb[:, 1:2], vb[:, 2:3], magic_sb[0:C, :])
        a_c = small.tile([C, 1], F32, tag="a_c")
        nc.vector.tensor_mul(a_c[:], gval_sb[:], vb[:, 1:2])
        b_c = small.tile([C, 1], F32, tag="b_c")
        nc.vector.tensor_mul(vb[:, 3:4], chan[:, 0:1], a_c[:])
        nc.vector.tensor_sub(b_c[:], bval_sb[:], vb[:, 3:4])

        out_sb = big_p.tile([C, NB, L], F32, tag="outsb")
        for g in range(4):
            nc.scalar.activation(
                out_sb[:, 2 * g : 2 * g + 2, :],
                ue_sb[:, 2 * g : 2 * g + 2, :],
                AF.Gelu,
                scale=a_c[:, 0:1],
                bias=b_c[:, 0:1],
            )
            for b in (2 * g, 2 * g + 1):
                nc.sync.dma_start(out=out_d[b], in_=out_sb[:, b, :])

    n = _dedupe_ldweights(nc)
    print(f"deduped {n} ldweights", file=sys.stderr)
    nc.compile()
    return nc


def make_in_map(q, k, v, gamma_sim, beta_sim, gamma_val, beta_val, h):
    """Build the per-core (per-head) input map. Layout-only host prep."""
    qh = q[:, h]
    kh = k[:, h]
    vh = v[:, h]

    import ml_dtypes

    def two(x):
        # [p = r*64+d, pair, l] <- x[2*pair+r, d, l]
        return np.ascontiguousarray(
            x.reshape(4, 2, 64, L).transpose(1, 2, 0, 3).reshape(128, 4, L)
        ).astype(np.float16)

    def t1(x, ones_col):
        # [p, b, c, j] <- X1[b, l=c*128+p, j], ones column at `ones_col`
        out = np.ones((NB, L, 65), dtype=np.float32)
        if ones_col == 64:
            out[:, :, :64] = x.transpose(0, 2, 1)
        else:
            out[:, :, 1:65] = x.transpose(0, 2, 1)
        return np.ascontiguousarray(
            out.reshape(NB, NCH, 128, 65).transpose(2, 0, 1, 3)
        ).astype(np.float16)

    return {
        "q2": two(qh),
        "k2": two(kh),
        "qt1": t1(qh, 64),
        "kt1": t1(kh, 64),
        "vt1": t1(vh, 64).astype(ml_dtypes.bfloat16),
        "g_sim": np.asarray(gamma_sim[h], dtype=np.float32).reshape(1, 1),
        "gamma_val": np.asarray(
            gamma_val[h * C : (h + 1) * C], dtype=np.float32
        ).reshape(C, 1),
        "beta_val": np.asarray(
            beta_val[h * C : (h + 1) * C], dtype=np.float32
        ).reshape(C, 1),
    }



_CACHED_NC = None


def _setup_profiling():
    """Make run_bass_kernel_spmd(trace=True) work on images missing
    antenv.axon_hooks: inject the ctypes NTFF hook + keep artifacts local."""
    import contextlib
    import ctypes
    import types

    try:
        from antenv.axon_hooks import get_axon_ntff_profile_hook  # noqa: F401
    except ImportError:
        so_path = os.environ.get("AXON_PJRT_SO", "/opt/axon/libaxon_pjrt.so")
        lib = ctypes.CDLL(so_path)
        lib.axon_start_nrt_profile.argtypes = [
            ctypes.POINTER(ctypes.c_int64),
            ctypes.c_size_t,
        ]
        lib.axon_start_nrt_profile.restype = ctypes.c_int64
        lib.axon_stop_nrt_profile.argtypes = [ctypes.c_char_p]
        lib.axon_stop_nrt_profile.restype = ctypes.c_int64

        @contextlib.contextmanager
        def _hook(output_dir, device_ids):
            import jax

            jax.devices()
            if device_ids:
                ids = (ctypes.c_int64 * len(device_ids))(*device_ids)
                rc = lib.axon_start_nrt_profile(ids, len(device_ids))
            else:
                rc = lib.axon_start_nrt_profile(None, 0)
            if rc != 0:
                raise RuntimeError(f"axon_start_nrt_profile rc={rc}")
            try:
                yield
            finally:
                n = lib.axon_stop_nrt_profile(str(output_dir).encode())
                print(f"ntff profile: {n} file(s) -> {output_dir}", file=sys.stderr)

        mod = types.ModuleType("antenv.axon_hooks")
        mod.get_axon_ntff_profile_hook = lambda: _hook
        mod.set_axon_ntff_profile_hook = lambda h: None
        import antenv

        sys.modules["antenv.axon_hooks"] = mod
        antenv.axon_hooks = mod

    import concourse.bass_utils as bu

    bu.upload_artifacts = lambda tmpdir: f"local://{tmpdir}"


def kernel(query, key, value, gamma_sim, beta_sim, gamma_val, beta_val):
    global _CACHED_NC
    from concourse.bass_utils import run_bass_kernel_spmd

    query = np.asarray(query, dtype=np.float32)
    key = np.asarray(key, dtype=np.float32)
    value = np.asarray(value, dtype=np.float32)
    gamma_sim = np.asarray(gamma_sim, dtype=np.float32)
    gamma_val = np.asarray(gamma_val, dtype=np.float32)
    beta_val = np.asarray(beta_val, dtype=np.float32)

    if _CACHED_NC is None:
        _CACHED_NC = build_nc()
    nc = _CACHED_NC

    in_maps = [
        make_in_map(query, key, value, gamma_sim, None, gamma_val, beta_val, h)
        for h in range(8)
    ]
    trace = bool(int(os.environ.get("BASS_PROFILE", "0")))
    tmpdir = os.environ.get("BASS_PROFILE_DIR") or None
    if trace:
        try:
            _setup_profiling()
        except Exception as e:  # noqa: BLE001
            print(f"profiling setup failed ({e}); running untraced", file=sys.stderr)
            trace = False
    try:
        res = run_bass_kernel_spmd(
            nc, in_maps, list(range(8)), trace=trace, tmpdir=tmpdir
        )
    except Exception:
        if not trace:
            raise
        print("traced run failed; retrying untraced", file=sys.stderr)
        res = run_bass_kernel_spmd(nc, in_maps, list(range(8)), trace=False)
    if trace and res.exec_time_ns is not None:
        print(f"HW exec time: {res.exec_time_ns} ns")

    out = np.empty((NB, 8 * C, L), dtype=np.float32)
    for h in range(8):
        out[:, h * C : (h + 1) * C, :] = res.results[h]["out"]
    return out


# revision 27
# speedup vs baseline: 1.1725x; 1.1725x over previous
"""Trainium2 Bass kernel for nn_AttentionOperation_32521492365427.

kernel(**inputs) -> np.ndarray, full shapes:
  query/key/value: [8, 8, 64, 1024] f32; gamma_sim/beta_sim: [8];
  gamma_val/beta_val: [512]; output: [8, 512, 1024] f32.

Sharded by HEAD across the 8 NeuronCores (one head per core): both
BatchNorms then have core-local statistics, so there are no collectives.

Per-core math:
 - softmax is shift-invariant => the sim-BN reduces to one per-head scale
   s = gamma_sim / sqrt(var(logits) + EPS); beta/mean drop out.
 - sumsq(logits_b) = sum(Gq_b * Gk_b) over 65x65 Gram matrices
   Gq = Q1^T Q1 (ones column appended) => logits variance without a
   stats pass over the 8.4M logits.
 - softmax denominators come free as row 0 of the PV matmul by having a
   ones column at position 0 of V^T (the stationary operand).
 - val-BN affine + exact (erf) gelu fuse into a single ACT pass.

v2 changes vs baseline:
 - all matmul operands fp16 (1 cyc/row like bf16, 4x the mantissa);
   host pre-converts so no DMA dtype conversion.
 - denominator broadcast via gpsimd partition_broadcast instead of an
   fp32 ones-matmul (fp32 matmuls are 4 cyc/row and burn PSUM banks).
 - all inputs SBUF-resident; per-batch DMA slices overlap phase 0.
 - PSUM: 2x lg [128,1024] + 2x pv [65,1024] = all 8 banks.
"""

import os
import sys

sys.path.insert(0, "/opt/trn_rl_repo")

from contextlib import ExitStack

import numpy as np

import concourse.bacc as bacc
import concourse.bass as bass  # noqa: F401
import concourse.tile as tile
from concourse import mybir

F32 = mybir.dt.float32
F16 = mybir.dt.float16
BF16 = mybir.dt.bfloat16
I32 = mybir.dt.int32
I16 = mybir.dt.int16
AF = mybir.ActivationFunctionType
OP = mybir.AluOpType

EPS = 1e-3
NB = 8
D = 64
C = 64
L = 1024
M = 1024
NCH = M // 128
NLM = float(NB * L * M)
MAGIC = 0x5F3759DF
# DVE Schraudolph exp: bf16 bits = int16(x * 128/ln2 + 16250.49); ~3% max rel
# err, exact dynamic range to e^+-88. Offloaded chunks relieve the ACT engine.
SCHRAU_A = 184.6635
SCHRAU_B = 16250.49
DVE_EXP_CHUNKS = (5,)


def _newton_rsqrt(nc, x, y, t, magic_i32, iters=1):
    """y = 1/sqrt(x) entirely on DVE (bit-trick seed + Newton iters)."""
    xi = x.bitcast(I32)
    yi = y.bitcast(I32)
    nc.vector.tensor_scalar(
        out=yi, in0=xi, scalar1=1, scalar2=None, op0=OP.arith_shift_right
    )
    nc.vector.tensor_tensor(out=yi, in0=magic_i32, in1=yi, op=OP.subtract)
    for _ in range(iters):
        nc.vector.tensor_mul(t, y, y)
        nc.vector.tensor_mul(t, t, x)
        nc.vector.tensor_scalar(
            out=t, in0=t, scalar1=-0.5, scalar2=1.5, op0=OP.mult, op1=OP.add
        )
        nc.vector.tensor_mul(y, y, t)


def _dedupe_ldweights(nc):
    """Remove back-to-back InstLdweights whose weights AP is identical to the
    one already resident in the PE array (warmup repeats and the QK/PV
    j-pairs). Waits on a removed load are merged into its matmul."""
    removed = 0
    for bb in nc.m.functions[0].blocks:
        insts = list(bb.instructions)
        out = []
        last_sig = None
        k = 0
        while k < len(insts):
            inst = insts[k]
            tname = type(inst).__name__
            if tname == "InstLdweights":
                ap = inst.ins[0]
                sig = (
                    str(ap),
                    getattr(inst, "perf_mode", None),
                    getattr(inst, "is_transpose", None),
                    getattr(inst, "tile_position", None),
                )
                nxt = insts[k + 1] if k + 1 < len(insts) else None
                if (
                    sig == last_sig
                    and nxt is not None
                    and type(nxt).__name__ == "InstMatmult"
                ):
                    si = inst.sync_info
                    if si is not None and si.on_wait:
                        nsi = nxt.sync_info
                        if nsi is None:
                            nxt.sync_info = si
                        else:
                            nsi.on_wait.extend(si.on_wait)
                            nxt.sync_info = nsi
                    if si is not None and si.on_update:
                        nsi = nxt.sync_info
                        nsi.on_update.extend(si.on_update)
                        nxt.sync_info = nsi
                    removed += 1
                    k += 1
                    continue
                last_sig = sig
            elif tname == "InstMatmult":
                pass  # matmuls do not disturb the loaded stationary
            elif getattr(inst, "engine", None) == mybir.EngineType.PE and not getattr(
                inst, "is_sequencer_only", False
            ):
                last_sig = None  # unknown PE instruction: assume clobbered
            out.append(inst)
            k += 1
        if removed:
            bb.instructions = out
    return removed


def build_nc(debug: bool = False):
    nc = bacc.Bacc("TRN2", target_bir_lowering=False, debug=debug)

    q2_d = nc.dram_tensor("q2", [128, NB // 2, L], F16, kind="ExternalInput")
    k2_d = nc.dram_tensor("k2", [128, NB // 2, L], F16, kind="ExternalInput")
    qt1_d = nc.dram_tensor("qt1", [128, NB, NCH, 65], F16, kind="ExternalInput")
    kt1_d = nc.dram_tensor("kt1", [128, NB, NCH, 65], F16, kind="ExternalInput")
    vt1_d = nc.dram_tensor("vt1", [128, NB, NCH, 65], BF16, kind="ExternalInput")
    gsim_d = nc.dram_tensor("g_sim", [1, 1], F32, kind="ExternalInput")
    gval_d = nc.dram_tensor("gamma_val", [C, 1], F32, kind="ExternalInput")
    bval_d = nc.dram_tensor("beta_val", [C, 1], F32, kind="ExternalInput")
    out_d = nc.dram_tensor("out", [NB, C, L], F32, kind="ExternalOutput")

    with tile.TileContext(nc) as tc, ExitStack() as ctx:
        const_p = ctx.enter_context(tc.tile_pool(name="const", bufs=1))
        pt_p = ctx.enter_context(tc.tile_pool(name="pt", bufs=3))
        dn_p = ctx.enter_context(tc.tile_pool(name="dn", bufs=2))
        big_p = ctx.enter_context(tc.tile_pool(name="big", bufs=1))
        small = ctx.enter_context(tc.tile_pool(name="small", bufs=1))

        # ---- constants + resident inputs ----
        ones65 = const_p.tile([65, 1], F32, tag="ones65")
        nc.vector.memset(ones65[:], 1.0)
        warm_w = const_p.tile([128, 128], F16, tag="warmw")
        nc.vector.memset(warm_w[:], 1.0)
        warm_m = const_p.tile([128, 512], F16, tag="warmm")
        nc.vector.memset(warm_m[:], 1.0)
        magic_sb = const_p.tile([65, 1], I32, tag="magic")
        nc.vector.memset(magic_sb[:], MAGIC)
        gsim_sb = const_p.tile([1, 1], F32, tag="gsim")
        nc.sync.dma_start(out=gsim_sb[:], in_=gsim_d[:])
        gval_sb = const_p.tile([C, 1], F32, tag="gval")
        nc.sync.dma_start(out=gval_sb[:], in_=gval_d[:])
        bval_sb = const_p.tile([C, 1], F32, tag="bval")
        nc.sync.dma_start(out=bval_sb[:], in_=bval_d[:])

        qt_sb = const_p.tile([128, NB, NCH, 65], F16, tag="qt")
        kt_sb = const_p.tile([128, NB, NCH, 65], F16, tag="kt")
        vt_sb = const_p.tile([128, NB, NCH, 65], BF16, tag="vt")
        q2_sb = const_p.tile([128, NB // 2, L], F16, tag="q2")
        k2_sb = const_p.tile([128, NB // 2, L], F16, tag="k2")
        # per-batch slices so phase-0 consumers start as data lands
        for b in range(NB):
            nc.gpsimd.dma_start(out=qt_sb[:, b], in_=qt1_d[:, b])
            nc.sync.dma_start(out=kt_sb[:, b], in_=kt1_d[:, b])
        for p in range(NB // 2):
            nc.scalar.dma_start(out=q2_sb[:, p, :], in_=q2_d[:, p, :])
            nc.sync.dma_start(out=k2_sb[:, p, :], in_=k2_d[:, p, :])
        for b in range(NB):
            nc.gpsimd.dma_start(out=vt_sb[:, b], in_=vt1_d[:, b])

        s_bcast = small.tile([128, 1], F32, tag="sbc")

        # ---- phase 0: Gram-matrix logits variance -> s ----
        with tc.tile_pool(name="gram", bufs=2, space="PSUM") as gram_p, tc.tile_pool(
            name="sf", bufs=1, space="PSUM"
        ) as sf_p, tc.tile_pool(name="warm", bufs=1, space="PSUM") as warm_p:
            wps = warm_p.tile([128, 512], F32, tag="warm")
            for _ in range(6):
                nc.tensor.matmul(wps[:], warm_w[:], warm_m[:], start=True, stop=True)

            acc = small.tile([65, NB], F32, tag="acc")
            for b in range(NB):
                gq_ps = gram_p.tile([65, 65], F32, tag="gq")
                gk_ps = gram_p.tile([65, 65], F32, tag="gk")
                for c in range(NCH):
                    nc.tensor.matmul(
                        gq_ps[:],
                        qt_sb[:, b, c, :],
                        qt_sb[:, b, c, :],
                        start=(c == 0),
                        stop=(c == NCH - 1),
                    )
                for c in range(NCH):
                    nc.tensor.matmul(
                        gk_ps[:],
                        kt_sb[:, b, c, :],
                        kt_sb[:, b, c, :],
                        start=(c == 0),
                        stop=(c == NCH - 1),
                    )
                gq_sb = small.tile([65, 65], F32, tag="gq_sb")
                nc.vector.tensor_copy(gq_sb[:], gq_ps[:])
                prod = small.tile([65, 65], F32, tag="prod")
                nc.vector.tensor_mul(prod[:], gq_sb[:], gk_ps[:])
                nc.vector.reduce_sum(
                    acc[:, b : b + 1], prod[:, 0:64], axis=mybir.AxisListType.X
                )

            red = small.tile([65, 1], F32, tag="red")
            nc.vector.reduce_sum(red[:], acc[:], axis=mybir.AxisListType.X)
            rhs65 = small.tile([65, 1], F32, tag="rhs65")
            nc.vector.tensor_scalar_mul(rhs65[0:64, :], red[0:64, :], 1.0 / NLM)
            nc.vector.tensor_scalar_mul(rhs65[64:65, :], red[64:65, :], 1.0 / NLM)
            nc.vector.tensor_mul(rhs65[64:65, :], rhs65[64:65, :], rhs65[64:65, :])
            nc.vector.tensor_scalar_mul(rhs65[64:65, :], rhs65[64:65, :], -1.0)
            var_ps = sf_p.tile([1, 1], F32, tag="var")
            nc.tensor.matmul(
                var_ps[:], ones65[:], rhs65[:], start=True, stop=True
            )
            sv = small.tile([1, 6], F32, tag="sv")
            nc.vector.tensor_scalar_add(sv[:, 0:1], var_ps[:], EPS)
            _newton_rsqrt(nc, sv[:, 0:1], sv[:, 1:2], sv[:, 2:3], magic_sb[0:1, :])
            nc.vector.tensor_mul(sv[:, 3:4], sv[:, 1:2], gsim_sb[:])
            s_src = small.tile([1, 1], F32, tag="s_src")
            nc.vector.tensor_copy(s_src[:], sv[:, 3:4])
            nc.gpsimd.partition_broadcast(s_bcast[:], s_src[:])
        sa_bcast = small.tile([128, 1], F32, tag="sabc")
        nc.vector.tensor_scalar_mul(sa_bcast[:], s_bcast[:], SCHRAU_A)

        # ---- phase A: QK -> exp -> PV -> normalize + stats ----
        # Software-pipelined: PE stream per slot is [PV(s), QK(s+3)] so the
        # tensor engine never waits on the exp of the current chunk.
        ue_sb = big_p.tile([C, NB, L], F32, tag="ue")
        stats = small.tile([C, NB * 2 * 6], F32, tag="stats")

        with tc.tile_pool(name="lg", bufs=3, space="PSUM") as lg_p, tc.tile_pool(
            name="pv", bufs=1, space="PSUM"
        ) as pv_p, tc.tile_pool(name="rv", bufs=2) as rv_p:
            lg_tiles = {}
            pt_tiles = {}
            pv_tiles = {}
            rv_tiles = {}
            dn_tiles = {}
            pending = {}
            NS = NB * NCH

            def at_slot(s, fn):
                pending.setdefault(s, []).append(fn)

            def emit_qk(s):
                b, c = divmod(s, NCH)
                pair, r = divmod(b, 2)
                rs = slice(r * 64, r * 64 + 64)
                lg = lg_p.tile([128, L], F32, tag="lg")
                for j in range(2):
                    nc.tensor.matmul(
                        lg[:, j * 512 : (j + 1) * 512],
                        k2_sb[rs, pair, c * 128 : (c + 1) * 128],
                        q2_sb[rs, pair, j * 512 : (j + 1) * 512],
                        start=True,
                        stop=True,
                    )
                lg_tiles[s] = lg

            def emit_exp(s):
                c = s % NCH
                lg = lg_tiles.pop(s)
                pt = pt_p.tile([128, L], BF16, tag="pt")
                if c in DVE_EXP_CHUNKS:
                    nc.vector.tensor_scalar(
                        out=pt.bitcast(I16)[:],
                        in0=lg[:],
                        scalar1=sa_bcast[:, 0:1],
                        scalar2=SCHRAU_B,
                        op0=OP.mult,
                        op1=OP.add,
                    )
                else:
                    nc.scalar.activation(pt[:], lg[:], AF.Exp, scale=s_bcast[:, 0:1])
                pt_tiles[s] = pt

            # staged pv drain: rv-copy/den row right away (frees pv), the
            # rest spread over later slots so DVE keeps serving the
            # schraudolph chunks without queueing behind the whole chain.
            def drain_a(b):
                pv = pv_tiles.pop(b)
                rv = rv_p.tile([C, L], F32, tag="rv")
                nc.scalar.copy(rv[:], pv[0:C, :])
                rv_tiles[b] = rv
                den0 = dn_p.tile([1, L], F32, tag="den0")
                nc.vector.tensor_copy(den0[:], pv[64:65, :])
                dn_tiles[b] = den0

            def drain_b(b):
                den0 = dn_tiles.pop(b)
                dnb = dn_p.tile([C, L], F32, tag="dnb")
                nc.gpsimd.partition_broadcast(dnb[:], den0[:])
                rec = dn_p.tile([C, L], F32, tag="rec")
                nc.vector.reciprocal_approx_fast(out=rec[:], in_=dnb[:])
                dn_tiles[b] = rec

            def drain_c(b):
                rec = dn_tiles.pop(b)
                rv = rv_tiles.pop(b)
                nc.vector.tensor_mul(ue_sb[:, b, :], rv[:], rec[:])

            def drain_d(b):
                for half in range(2):
                    nc.vector.bn_stats(
                        stats[:, (b * 2 + half) * 6 : (b * 2 + half + 1) * 6],
                        ue_sb[:, b, half * 512 : (half + 1) * 512],
                    )

            for s in range(3):
                emit_qk(s)
            emit_exp(0)
            for s in range(NS):
                b, c = divmod(s, NCH)
                if s + 1 < NS:
                    emit_exp(s + 1)
                if c == 0:
                    pv_tiles[b] = pv_p.tile([65, L], F32, tag="pv", name="pv")
                pv = pv_tiles[b]
                pt = pt_tiles.pop(s)
                for j in range(2):
                    nc.tensor.matmul(
                        pv[:, j * 512 : (j + 1) * 512],
                        vt_sb[:, b, c, :],
                        pt[:, j * 512 : (j + 1) * 512],
                        start=(c == 0),
                        stop=(c == NCH - 1),
                        skip_group_check=True,
                    )
                if s + 3 < NS:
                    emit_qk(s + 3)
                if c == NCH - 1:
                    at_slot(s, lambda bb=b: drain_a(bb))
                    at_slot(s, lambda bb=b: drain_b(bb))
                    at_slot(s, lambda bb=b: drain_c(bb))
                    at_slot(s, lambda bb=b: drain_d(bb))
                for fn in pending.pop(s, []):
                    fn()
            for s in sorted(pending):
                for fn in pending[s]:
                    fn()

        # ---- phase B: val-BN affine + gelu + store ----
        chan = small.tile([C, 2], F32, tag="chan")
        nc.vector.bn_aggr(chan[:], stats[:])
        vb = small.tile([C, 6], F32, tag="vb")
        nc.vector.tensor_scalar_add(vb[:, 0:1], chan[:, 1:2], EPS)
        _newton_rsqrt(nc, vb[:, 0:1], vb[:, 1:2], vb[:, 2:3], magic_sb[0:C, :])
        a_c = small.tile([C, 1], F32, tag="a_c")
        nc.vector.tensor_mul(a_c[:], gval_sb[:], vb[:, 1:2])
        b_c = small.tile([C, 1], F32, tag="b_c")
        nc.vector.tensor_mul(vb[:, 3:4], chan[:, 0:1], a_c[:])
        nc.vector.tensor_sub(b_c[:], bval_sb[:], vb[:, 3:4])

        out_sb = big_p.tile([C, NB, L], F32, tag="outsb")
        for g in range(4):
            nc.scalar.activation(
                out_sb[:, 2 * g : 2 * g + 2, :],
                ue_sb[:, 2 * g : 2 * g + 2, :],
                AF.Gelu,
                scale=a_c[:, 0:1],
                bias=b_c[:, 0:1],
            )
            for b in (2 * g, 2 * g + 1):
                nc.sync.dma_start(out=out_d[b], in_=out_sb[:, b, :])

    n = _dedupe_ldweights(nc)
    print(f"deduped {n} ldweights", file=sys.stderr)
    nc.compile()
    return nc


def make_in_map(q, k, v, gamma_sim, beta_sim, gamma_val, beta_val, h):
    """Build the per-core (per-head) input map. Layout-only host prep."""
    qh = q[:, h]
    kh = k[:, h]
    vh = v[:, h]

    import ml_dtypes

    def two(x):
        # [p = r*64+d, pair, l] <- x[2*pair+r, d, l]
        return np.ascontiguousarray(
            x.reshape(4, 2, 64, L).transpose(1, 2, 0, 3).reshape(128, 4, L)
        ).astype(np.float16)

    def t1(x, ones_col):
        # [p, b, c, j] <- X1[b, l=c*128+p, j], ones column at `ones_col`
        out = np.ones((NB, L, 65), dtype=np.float32)
        if ones_col == 64:
            out[:, :, :64] = x.transpose(0, 2, 1)
        else:
            out[:, :, 1:65] = x.transpose(0, 2, 1)
        return np.ascontiguousarray(
            out.reshape(NB, NCH, 128, 65).transpose(2, 0, 1, 3)
        ).astype(np.float16)

    return {
        "q2": two(qh),
        "k2": two(kh),
        "qt1": t1(qh, 64),
        "kt1": t1(kh, 64),
        "vt1": t1(vh, 64).astype(ml_dtypes.bfloat16),
        "g_sim": np.asarray(gamma_sim[h], dtype=np.float32).reshape(1, 1),
        "gamma_val": np.asarray(
            gamma_val[h * C : (h + 1) * C], dtype=np.float32
        ).reshape(C, 1),
        "beta_val": np.asarray(
            beta_val[h * C : (h + 1) * C], dtype=np.float32
        ).reshape(C, 1),
    }



_CACHED_NC = None


def _setup_profiling():
    """Make run_bass_kernel_spmd(trace=True) work on images missing
    antenv.axon_hooks: inject the ctypes NTFF hook + keep artifacts local."""
    import contextlib
    import ctypes
    import types

    try:
        from antenv.axon_hooks import get_axon_ntff_profile_hook  # noqa: F401
    except ImportError:
        so_path = os.environ.get("AXON_PJRT_SO", "/opt/axon/libaxon_pjrt.so")
        lib = ctypes.CDLL(so_path)
        lib.axon_start_nrt_profile.argtypes = [
            ctypes.POINTER(ctypes.c_int64),
            ctypes.c_size_t,
        ]
        lib.axon_start_nrt_profile.restype = ctypes.c_int64
        lib.axon_stop_nrt_profile.argtypes = [ctypes.c_char_p]
        lib.axon_stop_nrt_profile.restype = ctypes.c_int64

        @contextlib.contextmanager
        def _hook(output_dir, device_ids):
            import jax

            jax.devices()
            if device_ids:
                ids = (ctypes.c_int64 * len(device_ids))(*device_ids)
                rc = lib.axon_start_nrt_profile(ids, len(device_ids))
            else:
                rc = lib.axon_start_nrt_profile(None, 0)
            if rc != 0:
                raise RuntimeError(f"axon_start_nrt_profile rc={rc}")
            try:
                yield
            finally:
                n = lib.axon_stop_nrt_profile(str(output_dir).encode())
                print(f"ntff profile: {n} file(s) -> {output_dir}", file=sys.stderr)

        mod = types.ModuleType("antenv.axon_hooks")
        mod.get_axon_ntff_profile_hook = lambda: _hook
        mod.set_axon_ntff_profile_hook = lambda h: None
        import antenv

        sys.modules["antenv.axon_hooks"] = mod
        antenv.axon_hooks = mod

    import concourse.bass_utils as bu

    bu.upload_artifacts = lambda tmpdir: f"local://{tmpdir}"


def kernel(query, key, value, gamma_sim, beta_sim, gamma_val, beta_val):
    global _CACHED_NC
    from concourse.bass_utils import run_bass_kernel_spmd

    query = np.asarray(query, dtype=np.float32)
    key = np.asarray(key, dtype=np.float32)
    value = np.asarray(value, dtype=np.float32)
    gamma_sim = np.asarray(gamma_sim, dtype=np.float32)
    gamma_val = np.asarray(gamma_val, dtype=np.float32)
    beta_val = np.asarray(beta_val, dtype=np.float32)

    if _CACHED_NC is None:
        _CACHED_NC = build_nc()
    nc = _CACHED_NC

    in_maps = [
        make_in_map(query, key, value, gamma_sim, None, gamma_val, beta_val, h)
        for h in range(8)
    ]
    trace = bool(int(os.environ.get("BASS_PROFILE", "0")))
    tmpdir = os.environ.get("BASS_PROFILE_DIR") or None
    if trace:
        try:
            _setup_profiling()
        except Exception as e:  # noqa: BLE001
            print(f"profiling setup failed ({e}); running untraced", file=sys.stderr)
            trace = False
    try:
        res = run_bass_kernel_spmd(
            nc, in_maps, list(range(8)), trace=trace, tmpdir=tmpdir
        )
    except Exception:
        if not trace:
            raise
        print("traced run failed; retrying untraced", file=sys.stderr)
        res = run_bass_kernel_spmd(nc, in_maps, list(range(8)), trace=False)
    if trace and res.exec_time_ns is not None:
        print(f"HW exec time: {res.exec_time_ns} ns")

    out = np.empty((NB, 8 * C, L), dtype=np.float32)
    for h in range(8):
        out[:, h * C : (h + 1) * C, :] = res.results[h]["out"]
    return out
